# revision 49
# baseline (speedup 1.0000x reference)
"""Trainium2 kernel for nn_AdaptiveMetaLearnerV1.

The reference network applies two stacked LayerNorm-LSTM cells (with h0=c0=0,
so the recurrent path is a constant) independently to each of the P*B scalar
inputs.  The whole computation therefore folds into two scalar->scalar
functions f, g with
    x_out[n] = f(x[n])            (per element)
    qt_out   = mean_n g(x[n])     (single scalar)

Both f and g saturate for large |t| (LayerNorm is asymptotically
scale-invariant), so after the change of variables v = tanh(alpha * t) they
are low-degree polynomials in v.  The device kernel evaluates, per element:
    v  = tanh(alpha * t)                      (ScalarE activation)
    f  = poly_18(v),  g = poly_10(v)          (VectorE fused-Horner chain)
Per-partition sums of g ride the last g instruction's free accumulator; a
TensorE ones-matmul folds the 128 partials into one value so the qt output
is a single-descriptor DMA.  Polynomial coefficients are fit on the host
from the actual weight tensors (least squares on a Chebyshev-node grid in
v-space) and baked into the instruction immediates; the compiled graph is
cached per weight-set (and in the persistent neuron compile cache).

Sharding: data-parallel over the coordinate dimension P: core i processes
x[i*1250:(i+1)*1250, :] (80000 elements = [128 partitions x 625]).  Only
the qt mean needs a cross-core reduction, done on the host from the
per-core partial sums.

Custom VectorE ops (registered into concourse.dve_ops at import) fuse 3-4
Horner steps per instruction, e.g.
    S <- (((S + c_k) * v + c_{k-1}) * v + c_{k-2}) * v
so f+g cost 9 VectorE instructions total instead of ~60.  Input and output
DMAs are split across the two hardware-DGE queues (sync + scalar engines)
to halve descriptor-posting latency; the tanh activation table is preloaded
by a dummy activation while the input DMA is in flight.

Measured on TRN2 (neuron-profile): ~23.3 us NEFF exec per core,
x_out relnorm error ~2.3e-3, qt relative error ~1e-6.
"""

import functools
import os
import sys

import numpy as np

for _p in ("/opt/trn_rl_repo",):
    if _p not in sys.path and os.path.isdir(_p):
        sys.path.insert(0, _p)

# ---------------------------------------------------------------- constants
P, B, H, L = 10000, 64, 40, 2
N_CORES = 8
SHARD_P = P // N_CORES          # 1250 coordinates per core
SHARD_ELEMS = SHARD_P * B       # 80000 elements per core
PARTS = 128
FD = SHARD_ELEMS // PARTS       # 625 elements per partition
EPS = 1e-5

ALPHA = 0.50                    # v = tanh(ALPHA * t)
FIT_HI = 6.5                    # fit range in t
D_F = 12                        # f polynomial degree (F4 + 2*S3 + S2F)
D_G = 4                         # g polynomial degree (one F4A op)

_LAST_RESULTS = None            # test harness reads exec_time_ns from here


# ------------------------------------------------- custom fused-Horner ops
@functools.lru_cache(maxsize=1)
def _register_dve_ops():
    """Register the fused-Horner custom DVE ops in concourse.dve_ops.

    HORNER_F2_ANT : out = (v*c0 + c1) * v                 (chain start, 2 steps)
    HORNER_S2_ANT : out = ((S + c0) * v + c1) * v          (2 steps)
    HORNER_S1F_ANT: out = (S + c0) * v + c1                (1 step + final const)
    HORNER_S2A_ANT: HORNER_S2 with accum_out = sum(out)    (g-chain tail)
    HORNER_F3_ANT : out = ((v*c0 + c1)*v + c2) * v         (chain start, 3 steps)
    HORNER_S3_ANT : out = (((S+c0)*v + c1)*v + c2) * v     (3 steps)
    HORNER_S2F_ANT: out = ((S+c0)*v + c1)*v + c2           (2 steps + final const)
    HORNER_S3A_ANT: HORNER_S3 with accum_out = sum(out)    (g-chain tail)

    c2 rides the imm2 (compile-time literal) slot, so graphs using the
    3-step ops are specialized to one coefficient set.
    """
    from operator import add as _add

    from concourse import dve_ops
    from concourse.dve_spec import (
        C0,
        C1,
        C2,
        C3,
        Spec,
        Src0,
        Src1,
        Zero,
        _has_src1,
        _spill_c3_to_src1 as _spill,
        lower,
    )
    from concourse.dve_uop import DveOpSpec

    def _sum_ref(body_fn):
        def _r(in0, in1, s0, s1, imm2):
            b = body_fn(in0, in1, s0, s1, imm2).astype(np.float32)
            return b, b.reshape(b.shape[0], -1).sum(axis=-1, keepdims=True)

        return _r

    defs = [
        (
            "HORNER_F2_ANT",
            Spec(
                body=(Src0 * C0 + C1) * Src0,
                reference=lambda in0, in1, s0, s1, imm2: (
                    (in0.astype(np.float32) * s0 + s1) * in0
                ),
            ),
        ),
        (
            "HORNER_S2_ANT",
            Spec(
                body=((Src0 + C0) * Src1 + C1) * Src1,
                reference=lambda in0, in1, s0, s1, imm2: (
                    ((in0.astype(np.float32) + s0) * in1 + s1) * in1
                ),
            ),
        ),
        (
            "HORNER_S1F_ANT",
            Spec(
                body=(Src0 + C0) * Src1 + C1,
                reference=lambda in0, in1, s0, s1, imm2: (
                    (in0.astype(np.float32) + s0) * in1 + s1
                ),
            ),
        ),
        (
            "HORNER_S2A_ANT",
            Spec(
                body=((Src0 + C0) * Src1 + C1) * Src1,
                accum=_add,
                accum_init=Zero,
                reference=_sum_ref(
                    lambda in0, in1, s0, s1, imm2: (
                        ((in0.astype(np.float32) + s0) * in1 + s1) * in1
                    )
                ),
            ),
        ),
        (
            "HORNER_F3_ANT",
            Spec(
                body=((Src0 * C0 + C1) * Src0 + C2) * Src0,
                reference=lambda in0, in1, s0, s1, imm2: (
                    ((in0.astype(np.float32) * s0 + s1) * in0 + imm2) * in0
                ),
            ),
        ),
        (
            "HORNER_S3_ANT",
            Spec(
                body=(((Src0 + C0) * Src1 + C1) * Src1 + C2) * Src1,
                reference=lambda in0, in1, s0, s1, imm2: (
                    (((in0.astype(np.float32) + s0) * in1 + s1) * in1 + imm2)
                    * in1
                ),
            ),
        ),
        (
            "HORNER_S2F_ANT",
            Spec(
                body=((Src0 + C0) * Src1 + C1) * Src1 + C2,
                reference=lambda in0, in1, s0, s1, imm2: (
                    ((in0.astype(np.float32) + s0) * in1 + s1) * in1 + imm2
                ),
            ),
        ),
        (
            "HORNER_S3A_ANT",
            Spec(
                body=(((Src0 + C0) * Src1 + C1) * Src1 + C2) * Src1,
                accum=_add,
                accum_init=Zero,
                reference=_sum_ref(
                    lambda in0, in1, s0, s1, imm2: (
                        (((in0.astype(np.float32) + s0) * in1 + s1) * in1
                         + imm2) * in1
                    )
                ),
            ),
        ),
        (
            # 4-step chain start: the 4th constant rides Src1 (C3 spill),
            # which is otherwise unused by the single-stream start op.
            "HORNER_F4_ANT",
            Spec(
                body=_spill(
                    (((Src0 * C0 + C1) * Src0 + C2) * Src0 + C3) * Src0
                ),
                reference=lambda in0, in1, s0, s1, imm2: (
                    (((in0.astype(np.float32) * s0 + s1) * in0 + imm2) * in0
                     + in1) * in0
                ),
            ),
        ),
        (
            # F4 with sum accumulator: a whole degree-4 g-chain in one op
            "HORNER_F4A_ANT",
            Spec(
                body=_spill(
                    (((Src0 * C0 + C1) * Src0 + C2) * Src0 + C3) * Src0
                ),
                accum=_add,
                accum_init=Zero,
                reference=_sum_ref(
                    lambda in0, in1, s0, s1, imm2: (
                        (((in0.astype(np.float32) * s0 + s1) * in0 + imm2)
                         * in0 + in1) * in0
                    )
                ),
            ),
        ),
    ]

    ops = {}
    for name, spec in defs:
        existing = {op.name: op for op in dve_ops.OPS}
        if name in existing:
            ops[name] = existing[name]
            continue
        row = dve_ops._CUSTOM_DVE_ROW_BASE + len(dve_ops.OPS)
        assert row < 0x20, "custom-DVE row field overflow"
        shas = {}
        for ver in ("v3", "v4"):
            try:
                shas[ver] = DveOpSpec(
                    name=name,
                    opcode=row,
                    uops=lower(spec, ver=ver),
                    rd1_en=_has_src1(spec),
                ).sha(ver)
            except Exception:
                pass
        op = dve_ops.DveOp(name, spec, subdim=False, uops_sha=shas)
        dve_ops.OPS.append(op)
        dve_ops.CUSTOM_DVE_SPECS[name] = spec
        dve_ops._SUB_OPCODE_FOR_NAME[name] = row
        ops[name] = op
    return ops


# --------------------------------------------------- folded scalar network
def _ln(x, g, b):
    m = x.mean(-1, keepdims=True)
    v = ((x - m) ** 2).mean(-1, keepdims=True)
    return (x - m) / np.sqrt(v + EPS) * g + b


def _fold_scalar_fn(t, W1, b1, Wih, Whh, bih, bhh, g_ih, be_ih, g_hh, be_hh,
                    g_c, be_c, Wout, bout):
    """Evaluate the folded LSTM stack on scalar inputs t (f64 numpy).

    Uses h0 = c0 = 0: the Whh branch LayerNorm collapses to a constant and
    the forget gate drops out entirely.
    """
    x_in = t[:, None] * W1[:, 0][None, :] + b1[None, :]
    for l in range(L):
        hh_const = _ln(bhh[l][None, :], g_hh[l], be_hh[l])[0]
        gates = _ln(x_in @ Wih[l].T + bih[l], g_ih[l], be_ih[l]) + hh_const
        i, f, g, o = np.split(gates, 4, axis=-1)
        c = 1.0 / (1.0 + np.exp(-i)) * np.tanh(g)
        x_in = 1.0 / (1.0 + np.exp(-o)) * np.tanh(_ln(c, g_c[l], be_c[l]))
    return (x_in @ Wout.T + bout)[:, 0]


def _fit_poly_in_v(fn, alpha, deg):
    """Least-squares fit of fn(t) as a polynomial in v = tanh(alpha*t).

    Sample at Chebyshev nodes in v-space plus uniform t points; solve in
    float64, weighting samples toward the standard-normal bulk of the
    data (with a floor so the tails stay controlled).  Returns monomial
    coefficients c[0..deg] in raw (unnormalized) v, so the device
    evaluates sum_k c_k * tanh(alpha*t)**k directly.
    """
    vmax = np.tanh(alpha * FIT_HI)
    vn = np.cos(np.pi * (np.arange(2500) + 0.5) / 2500) * vmax
    t_nodes = np.arctanh(vn) / alpha
    t_unif = np.linspace(-FIT_HI, FIT_HI, 801)
    tg = np.concatenate([t_nodes, t_unif])
    vg = np.tanh(alpha * tg)
    w = np.exp(-0.25 * tg ** 2) + 0.05
    A = np.stack([vg ** k for k in range(deg + 1)], axis=1) * w[:, None]
    y = fn(tg) * w
    coef, _, _, _ = np.linalg.lstsq(A, y, rcond=None)
    return coef


@functools.lru_cache(maxsize=4)
def _fit_coeffs_cached(weights_key):
    d = dict(np.load(_fit_coeffs_cached._path))
    f_fn = lambda t: _fold_scalar_fn(
        t, d["W1"], d["b1"], d["Wih"], d["Whh"], d["bih"], d["bhh"],
        d["g_ih"], d["be_ih"], d["g_hh"], d["be_hh"], d["g_c"], d["be_c"],
        d["Wout"], d["bout"])
    lam = float(d["lambda_q"][0, 0])
    g_fn = lambda t: lam * np.tanh(_fold_scalar_fn(
        t, d["aW1"], d["ab1"], d["aWih"], d["aWhh"], d["abih"], d["abhh"],
        d["ag_ih"], d["abe_ih"], d["ag_hh"], d["abe_hh"], d["ag_c"],
        d["abe_c"], d["aWout"], d["about"]))
    cf = _fit_poly_in_v(f_fn, ALPHA, D_F)
    cg = _fit_poly_in_v(g_fn, ALPHA, D_G)
    return cf, cg


def _fit_coeffs(weights):
    """Cache the (slow-ish) host fit on the weight bytes."""
    import hashlib
    import tempfile

    h = hashlib.sha256()
    for k in sorted(weights):
        h.update(k.encode())
        h.update(np.ascontiguousarray(weights[k]).tobytes())
    key = h.hexdigest()
    path = os.path.join(tempfile.gettempdir(), f"aml_weights_{key}.npz")
    if not os.path.exists(path):
        np.savez(path, **weights)
    _fit_coeffs_cached._path = path
    return _fit_coeffs_cached(key)


# ------------------------------------------------------------- bass graph
@functools.lru_cache(maxsize=4)
def _build_graph(cf_t, cg_t):
    """Build the per-coefficient-set graph.

    cf_t: f coefficients as float tuple, highest degree first (c_18..c_0).
    cg_t: g coefficients, highest degree first (c_10..c_1) — c_0 is added
          on the host after the mean.

    Engine schedule: one Tanh on ScalarE (table preloaded via a dummy
    activation during the input DMA; the chain-start C3 constants are also
    written by ScalarE in that shadow), then VectorE runs the g chain
    (3 fused ops, per-partition sums via accum_out), TensorE ones-matmuls
    the 128 partials to one value for a single-descriptor DMA, and the
    f chain (6 fused ops) overlaps the q-path; x is DMA'd out at the end.
    Input and x-output DMAs are split across the two hardware-DGE queues
    (sync + scalar engines) to halve descriptor-posting latency.
    """
    import concourse.bass as bass
    import concourse.mybir as mybir

    ops = _register_dve_ops()
    F4, S3, S2F, F4A = (
        ops["HORNER_F4_ANT"],
        ops["HORNER_S3_ANT"],
        ops["HORNER_S2F_ANT"],
        ops["HORNER_F4A_ANT"],
    )

    f32 = mybir.dt.float32
    nc = bass.Bass()

    x_ext = nc.declare_dram_parameter("x", [SHARD_ELEMS], f32, isOutput=False)
    outx_ext = nc.declare_dram_parameter("out_x", [SHARD_ELEMS], f32, isOutput=True)
    outq_ext = nc.declare_dram_parameter("out_q", [1, 1], f32, isOutput=True)

    x_t = x_ext[:].rearrange("(p f) -> p f", p=PARTS)
    outx_t = outx_ext[:].rearrange("(p f) -> p f", p=PARTS)

    with (
        nc.sbuf_tensor([PARTS, FD], f32) as Tt,
        nc.sbuf_tensor([PARTS, FD], f32) as Vt,
        nc.sbuf_tensor([PARTS, FD], f32) as Sf,
        nc.sbuf_tensor([PARTS, FD], f32) as Sg,
        nc.sbuf_tensor([PARTS, 1], f32) as Qp,
        nc.psum_tensor([1, 1], f32) as Pq,
        nc.sbuf_tensor([1, 1], f32) as Qs,
        nc.sbuf_tensor([PARTS, 1], f32) as Dm,
        nc.sbuf_tensor([PARTS, 1], f32) as C3f,
        nc.sbuf_tensor([PARTS, 1], f32) as C3g,
        nc.semaphore("dma_sem") as dma_sem,
        nc.semaphore("act_sem") as act_sem,
        nc.semaphore("vec_sem") as vec_sem,
        nc.semaphore("gp_sem") as gp_sem,
        nc.Block() as block,
    ):
        HP = PARTS // 2
        h0, h1 = slice(0, HP), slice(HP, PARTS)

        @block.sync
        def _(sync):
            sync.dma_start(out=Tt[h0, :], in_=x_t[h0, :]).then_inc(dma_sem, 16)
            sync.wait_ge(vec_sem, 1)
            sync.dma_start(out=outx_t[h0, :], in_=Sf[h0, :]).then_inc(dma_sem, 16)
            sync.wait_ge(act_sem, 2)
            sync.dma_start(out=outq_ext[:], in_=Qs[:]).then_inc(dma_sem, 16)
            sync.wait_ge(dma_sem, 80)

        @block.scalar
        def _(scalar):
            scalar.dma_start(out=Tt[h1, :], in_=x_t[h1, :]).then_inc(dma_sem, 16)
            zero = nc.const_aps.scalar_like(0.0, Dm[:])
            one = nc.const_aps.scalar_like(1.0, Dm[:])
            # chain-start 4th constants (C3 spill operands), written while
            # the input DMA is in flight: out = Copy(c * 1)
            scalar.mul(C3f[:], one, float(cf_t[3]))
            scalar.mul(C3g[:], one, float(cg_t[3]))
            # dummy activation: triggers ACT_TABLE_LOAD for the tanh set
            scalar.activation(
                Dm[:], zero,
                mybir.ActivationFunctionType.Tanh, bias=0.0, scale=1.0,
            )
            scalar.wait_ge(dma_sem, 32)
            scalar.activation(
                Vt[:], Tt[:], mybir.ActivationFunctionType.Tanh,
                bias=0.0, scale=float(ALPHA),
            ).then_inc(act_sem, 1)
            scalar.wait_ge(vec_sem, 1)
            scalar.dma_start(out=outx_t[h1, :], in_=Sf[h1, :]).then_inc(dma_sem, 16)
            # q-sum PSUM -> SBUF; sync's idle HW queue ships it
            scalar.wait_ge(gp_sem, 1)
            scalar.copy(Qs[:], Pq[:]).then_inc(act_sem, 1)

        @block.vector
        def _(vector):
            vector.wait_ge(act_sem, 1)
            # ---- f chain first: its big output DMA posts while g runs ----
            vector._custom_dve(F4, out=Sf[:], in0=Vt[:], in1=C3f[:],
                               s0=float(cf_t[0]), s1=float(cf_t[1]),
                               imm2=float(cf_t[2]))
            for j in range(2):
                vector._custom_dve(
                    S3, out=Sf[:], in0=Sf[:], in1=Vt[:],
                    s0=float(cf_t[4 + 3 * j]), s1=float(cf_t[5 + 3 * j]),
                    imm2=float(cf_t[6 + 3 * j]),
                )
            # final: S = ((S + c2)*v + c1)*v + c0
            vector._custom_dve(
                S2F, out=Sf[:], in0=Sf[:], in1=Vt[:],
                s0=float(cf_t[10]), s1=float(cf_t[11]), imm2=float(cf_t[12]),
            ).then_inc(vec_sem, 1)

            # ---- g chain: degree D_G = 4, one op with sum accumulator ----
            vector._custom_dve(
                F4A, out=Sg[:], accum_out=Qp[:], in0=Vt[:], in1=C3g[:],
                s0=float(cg_t[0]), s1=float(cg_t[1]), imm2=float(cg_t[2]),
            ).then_inc(vec_sem, 1)

        @block.tensor
        def _(tensor):
            # sum the 128 per-partition partials with a ones-matmul:
            # Pq[0,0] = sum_k Qp[k,0] * 1
            tensor.wait_ge(vec_sem, 2)
            tensor.matmul(
                Pq[:], Qp[:], nc.const_aps.tensor(1.0, (PARTS, 1)),
            ).then_inc(gp_sem, 1)

    # raw Bass does not lower wrapper instructions (InstCustomDveAnt) to ISA
    # bytes on its own; Bacc.compile does this, so do it explicitly here.
    mybir.codegen_inst_isa_subclasses(nc)
    return nc


# ---------------------------------------------------------------- kernel
def kernel(**inputs):
    global _LAST_RESULTS
    from concourse.bass_utils import run_bass_kernel_spmd

    weights = {k: np.asarray(v, dtype=np.float32) for k, v in inputs.items()
               if k != "x"}
    x = np.asarray(inputs["x"], dtype=np.float32)
    assert x.shape == (P, B)

    cf64, cg64 = _fit_coeffs(weights)
    # device ordering: highest degree first; g's c_0 stays on the host
    cf_t = tuple(float(np.float32(v)) for v in cf64[::-1])          # c23..c0
    cg_t = tuple(float(np.float32(v)) for v in cg64[:0:-1])         # c12..c1

    nc = _build_graph(cf_t, cg_t)

    in_maps = []
    for i in range(N_CORES):
        shard = np.ascontiguousarray(
            x[i * SHARD_P:(i + 1) * SHARD_P, :]).reshape(-1)
        in_maps.append({"x": shard})

    res = run_bass_kernel_spmd(
        nc, in_maps, core_ids=list(range(N_CORES)),
        trace=bool(os.environ.get("AML_TRACE")),
    )
    _LAST_RESULTS = res

    x_out = np.concatenate(
        [np.asarray(res.results[i]["out_x"], dtype=np.float32)
         for i in range(N_CORES)]
    ).reshape(P * B, 1)

    qsum = np.float64(0.0)
    for i in range(N_CORES):
        qsum += float(np.asarray(res.results[i]["out_q"]).reshape(-1)[0])
    qt = np.float32(qsum / (P * B) + cg64[0])
    return x_out, np.array([qt], dtype=np.float32)


# revision 51
# speedup vs baseline: 1.0194x; 1.0194x over previous
"""Trainium2 kernel for nn_AdaptiveMetaLearnerV1.

The reference network applies two stacked LayerNorm-LSTM cells (with h0=c0=0,
so the recurrent path is a constant) independently to each of the P*B scalar
inputs.  The whole computation therefore folds into two scalar->scalar
functions f, g with
    x_out[n] = f(x[n])            (per element)
    qt_out   = mean_n g(x[n])     (single scalar)

Both f and g saturate for large |t| (LayerNorm is asymptotically
scale-invariant), so after the change of variables v = tanh(alpha * t) they
are low-degree polynomials in v.  The device kernel evaluates, per element:
    v  = tanh(alpha * t)                      (ScalarE activation)
    f  = poly_12(v),  g = poly_4(v)           (VectorE fused-Horner chain)
Per-partition sums of g ride the g instruction's free accumulator; a
TensorE ones-matmul folds the 128 partials into one value so the qt output
is a single-descriptor DMA.  Polynomial coefficients are fit on the host
from the actual weight tensors (weighted least squares on a Chebyshev-node
grid in v-space) and baked into the instruction immediates; the compiled
graph is cached per weight-set (and in the persistent neuron compile cache).

Sharding: data-parallel over the coordinate dimension P: core i processes
x[i*1250:(i+1)*1250, :] (80000 elements = [128 partitions x 625]).  Only
the qt mean needs a cross-core reduction, done on the host from the
per-core partial sums.

Custom VectorE ops (registered into concourse.dve_ops at import) fuse 3-4
Horner steps per instruction, e.g.
    S <- (((S + c_k) * v + c_{k-1}) * v + c_{k-2}) * v
(chain starts use the C3->Src1 spill for a 4th constant; the g op carries
the sum accumulator in its 8th ALU stage), so f+g cost 5 VectorE
instructions total instead of ~50.  The f chain runs first so its big
x-output DMA posts while the g/q path completes underneath; input and
x-output DMAs are split across the two hardware-DGE queues (sync + scalar
engines); the tanh activation table is preloaded by a dummy activation
while the input DMA is in flight.

Measured on TRN2 (neuron-profile): ~19.7-22.5 us NEFF exec per core
(bimodal with device state), x_out relnorm error 3.58e-3, qt relative
error 2.7e-4.
"""

import functools
import os
import sys

import numpy as np

for _p in ("/opt/trn_rl_repo",):
    if _p not in sys.path and os.path.isdir(_p):
        sys.path.insert(0, _p)

# ---------------------------------------------------------------- constants
P, B, H, L = 10000, 64, 40, 2
N_CORES = 8
SHARD_P = P // N_CORES          # 1250 coordinates per core
SHARD_ELEMS = SHARD_P * B       # 80000 elements per core
PARTS = 128
FD = SHARD_ELEMS // PARTS       # 625 elements per partition
EPS = 1e-5

ALPHA = 0.50                    # v = tanh(ALPHA * t)
FIT_HI = 6.5                    # fit range in t
D_F = 12                        # f polynomial degree (F4 + 2*S3 + S2F)
D_G = 4                         # g polynomial degree (one F4A op)

_LAST_RESULTS = None            # test harness reads exec_time_ns from here


# ------------------------------------------------- custom fused-Horner ops
@functools.lru_cache(maxsize=1)
def _register_dve_ops():
    """Register the fused-Horner custom DVE ops in concourse.dve_ops.

    HORNER_F2_ANT : out = (v*c0 + c1) * v                 (chain start, 2 steps)
    HORNER_S2_ANT : out = ((S + c0) * v + c1) * v          (2 steps)
    HORNER_S1F_ANT: out = (S + c0) * v + c1                (1 step + final const)
    HORNER_S2A_ANT: HORNER_S2 with accum_out = sum(out)    (g-chain tail)
    HORNER_F3_ANT : out = ((v*c0 + c1)*v + c2) * v         (chain start, 3 steps)
    HORNER_S3_ANT : out = (((S+c0)*v + c1)*v + c2) * v     (3 steps)
    HORNER_S2F_ANT: out = ((S+c0)*v + c1)*v + c2           (2 steps + final const)
    HORNER_S3A_ANT: HORNER_S3 with accum_out = sum(out)    (g-chain tail)

    c2 rides the imm2 (compile-time literal) slot, so graphs using the
    3-step ops are specialized to one coefficient set.
    """
    from operator import add as _add

    from concourse import dve_ops
    from concourse.dve_spec import (
        C0,
        C1,
        C2,
        C3,
        Spec,
        Src0,
        Src1,
        Zero,
        _has_src1,
        _spill_c3_to_src1 as _spill,
        lower,
    )
    from concourse.dve_uop import DveOpSpec

    def _sum_ref(body_fn):
        def _r(in0, in1, s0, s1, imm2):
            b = body_fn(in0, in1, s0, s1, imm2).astype(np.float32)
            return b, b.reshape(b.shape[0], -1).sum(axis=-1, keepdims=True)

        return _r

    defs = [
        (
            "HORNER_F2_ANT",
            Spec(
                body=(Src0 * C0 + C1) * Src0,
                reference=lambda in0, in1, s0, s1, imm2: (
                    (in0.astype(np.float32) * s0 + s1) * in0
                ),
            ),
        ),
        (
            "HORNER_S2_ANT",
            Spec(
                body=((Src0 + C0) * Src1 + C1) * Src1,
                reference=lambda in0, in1, s0, s1, imm2: (
                    ((in0.astype(np.float32) + s0) * in1 + s1) * in1
                ),
            ),
        ),
        (
            "HORNER_S1F_ANT",
            Spec(
                body=(Src0 + C0) * Src1 + C1,
                reference=lambda in0, in1, s0, s1, imm2: (
                    (in0.astype(np.float32) + s0) * in1 + s1
                ),
            ),
        ),
        (
            "HORNER_S2A_ANT",
            Spec(
                body=((Src0 + C0) * Src1 + C1) * Src1,
                accum=_add,
                accum_init=Zero,
                reference=_sum_ref(
                    lambda in0, in1, s0, s1, imm2: (
                        ((in0.astype(np.float32) + s0) * in1 + s1) * in1
                    )
                ),
            ),
        ),
        (
            "HORNER_F3_ANT",
            Spec(
                body=((Src0 * C0 + C1) * Src0 + C2) * Src0,
                reference=lambda in0, in1, s0, s1, imm2: (
                    ((in0.astype(np.float32) * s0 + s1) * in0 + imm2) * in0
                ),
            ),
        ),
        (
            "HORNER_S3_ANT",
            Spec(
                body=(((Src0 + C0) * Src1 + C1) * Src1 + C2) * Src1,
                reference=lambda in0, in1, s0, s1, imm2: (
                    (((in0.astype(np.float32) + s0) * in1 + s1) * in1 + imm2)
                    * in1
                ),
            ),
        ),
        (
            "HORNER_S2F_ANT",
            Spec(
                body=((Src0 + C0) * Src1 + C1) * Src1 + C2,
                reference=lambda in0, in1, s0, s1, imm2: (
                    ((in0.astype(np.float32) + s0) * in1 + s1) * in1 + imm2
                ),
            ),
        ),
        (
            "HORNER_S3A_ANT",
            Spec(
                body=(((Src0 + C0) * Src1 + C1) * Src1 + C2) * Src1,
                accum=_add,
                accum_init=Zero,
                reference=_sum_ref(
                    lambda in0, in1, s0, s1, imm2: (
                        (((in0.astype(np.float32) + s0) * in1 + s1) * in1
                         + imm2) * in1
                    )
                ),
            ),
        ),
        (
            # 4-step chain start: the 4th constant rides Src1 (C3 spill),
            # which is otherwise unused by the single-stream start op.
            "HORNER_F4_ANT",
            Spec(
                body=_spill(
                    (((Src0 * C0 + C1) * Src0 + C2) * Src0 + C3) * Src0
                ),
                reference=lambda in0, in1, s0, s1, imm2: (
                    (((in0.astype(np.float32) * s0 + s1) * in0 + imm2) * in0
                     + in1) * in0
                ),
            ),
        ),
        (
            # F4 with sum accumulator: a whole degree-4 g-chain in one op
            "HORNER_F4A_ANT",
            Spec(
                body=_spill(
                    (((Src0 * C0 + C1) * Src0 + C2) * Src0 + C3) * Src0
                ),
                accum=_add,
                accum_init=Zero,
                reference=_sum_ref(
                    lambda in0, in1, s0, s1, imm2: (
                        (((in0.astype(np.float32) * s0 + s1) * in0 + imm2)
                         * in0 + in1) * in0
                    )
                ),
            ),
        ),
    ]

    ops = {}
    for name, spec in defs:
        existing = {op.name: op for op in dve_ops.OPS}
        if name in existing:
            ops[name] = existing[name]
            continue
        row = dve_ops._CUSTOM_DVE_ROW_BASE + len(dve_ops.OPS)
        assert row < 0x20, "custom-DVE row field overflow"
        shas = {}
        for ver in ("v3", "v4"):
            try:
                shas[ver] = DveOpSpec(
                    name=name,
                    opcode=row,
                    uops=lower(spec, ver=ver),
                    rd1_en=_has_src1(spec),
                ).sha(ver)
            except Exception:
                pass
        op = dve_ops.DveOp(name, spec, subdim=False, uops_sha=shas)
        dve_ops.OPS.append(op)
        dve_ops.CUSTOM_DVE_SPECS[name] = spec
        dve_ops._SUB_OPCODE_FOR_NAME[name] = row
        ops[name] = op
    return ops


# --------------------------------------------------- folded scalar network
def _ln(x, g, b):
    m = x.mean(-1, keepdims=True)
    v = ((x - m) ** 2).mean(-1, keepdims=True)
    return (x - m) / np.sqrt(v + EPS) * g + b


def _fold_scalar_fn(t, W1, b1, Wih, Whh, bih, bhh, g_ih, be_ih, g_hh, be_hh,
                    g_c, be_c, Wout, bout):
    """Evaluate the folded LSTM stack on scalar inputs t (f64 numpy).

    Uses h0 = c0 = 0: the Whh branch LayerNorm collapses to a constant and
    the forget gate drops out entirely.
    """
    x_in = t[:, None] * W1[:, 0][None, :] + b1[None, :]
    for l in range(L):
        hh_const = _ln(bhh[l][None, :], g_hh[l], be_hh[l])[0]
        gates = _ln(x_in @ Wih[l].T + bih[l], g_ih[l], be_ih[l]) + hh_const
        i, f, g, o = np.split(gates, 4, axis=-1)
        c = 1.0 / (1.0 + np.exp(-i)) * np.tanh(g)
        x_in = 1.0 / (1.0 + np.exp(-o)) * np.tanh(_ln(c, g_c[l], be_c[l]))
    return (x_in @ Wout.T + bout)[:, 0]


def _fit_poly_in_v(fn, alpha, deg):
    """Least-squares fit of fn(t) as a polynomial in v = tanh(alpha*t).

    Sample at Chebyshev nodes in v-space plus uniform t points; solve in
    float64, weighting samples toward the standard-normal bulk of the
    data (with a floor so the tails stay controlled).  Returns monomial
    coefficients c[0..deg] in raw (unnormalized) v, so the device
    evaluates sum_k c_k * tanh(alpha*t)**k directly.
    """
    vmax = np.tanh(alpha * FIT_HI)
    vn = np.cos(np.pi * (np.arange(2500) + 0.5) / 2500) * vmax
    t_nodes = np.arctanh(vn) / alpha
    t_unif = np.linspace(-FIT_HI, FIT_HI, 801)
    tg = np.concatenate([t_nodes, t_unif])
    vg = np.tanh(alpha * tg)
    w = np.exp(-0.25 * tg ** 2) + 0.05
    A = np.stack([vg ** k for k in range(deg + 1)], axis=1) * w[:, None]
    y = fn(tg) * w
    coef, _, _, _ = np.linalg.lstsq(A, y, rcond=None)
    return coef


@functools.lru_cache(maxsize=4)
def _fit_coeffs_cached(weights_key):
    d = dict(np.load(_fit_coeffs_cached._path))
    f_fn = lambda t: _fold_scalar_fn(
        t, d["W1"], d["b1"], d["Wih"], d["Whh"], d["bih"], d["bhh"],
        d["g_ih"], d["be_ih"], d["g_hh"], d["be_hh"], d["g_c"], d["be_c"],
        d["Wout"], d["bout"])
    lam = float(d["lambda_q"][0, 0])
    g_fn = lambda t: lam * np.tanh(_fold_scalar_fn(
        t, d["aW1"], d["ab1"], d["aWih"], d["aWhh"], d["abih"], d["abhh"],
        d["ag_ih"], d["abe_ih"], d["ag_hh"], d["abe_hh"], d["ag_c"],
        d["abe_c"], d["aWout"], d["about"]))
    cf = _fit_poly_in_v(f_fn, ALPHA, D_F)
    cg = _fit_poly_in_v(g_fn, ALPHA, D_G)
    return cf, cg


def _fit_coeffs(weights):
    """Cache the (slow-ish) host fit on the weight bytes."""
    import hashlib
    import tempfile

    h = hashlib.sha256()
    for k in sorted(weights):
        h.update(k.encode())
        h.update(np.ascontiguousarray(weights[k]).tobytes())
    key = h.hexdigest()
    path = os.path.join(tempfile.gettempdir(), f"aml_weights_{key}.npz")
    if not os.path.exists(path):
        np.savez(path, **weights)
    _fit_coeffs_cached._path = path
    return _fit_coeffs_cached(key)


# ------------------------------------------------------------- bass graph
@functools.lru_cache(maxsize=4)
def _build_graph(cf_t, cg_t):
    """Build the per-coefficient-set graph.

    cf_t: f coefficients as float tuple, highest degree first (c_12..c_0).
    cg_t: g coefficients, highest degree first (c_4..c_1) — c_0 is added
          on the host after the mean.

    Engine schedule: one Tanh on ScalarE (table preloaded via a dummy
    activation during the input DMA; the chain-start C3 constants are also
    written by ScalarE in that shadow), then VectorE runs the f chain
    (4 fused ops) whose x-output DMA posts while the g op (1 fused op with
    per-partition sums via accum_out) and the q path complete underneath:
    TensorE ones-matmuls the 128 partials to one value, ScalarE copies it
    out of PSUM, and sync ships it as a single-descriptor DMA.  Input and
    x-output DMAs are split across the two hardware-DGE queues
    (sync + scalar engines) to halve descriptor-posting latency.
    """
    import concourse.bass as bass
    import concourse.mybir as mybir

    ops = _register_dve_ops()
    F4, S3, S2F, F4A = (
        ops["HORNER_F4_ANT"],
        ops["HORNER_S3_ANT"],
        ops["HORNER_S2F_ANT"],
        ops["HORNER_F4A_ANT"],
    )

    f32 = mybir.dt.float32
    nc = bass.Bass()

    x_ext = nc.declare_dram_parameter("x", [SHARD_ELEMS], f32, isOutput=False)
    outx_ext = nc.declare_dram_parameter("out_x", [SHARD_ELEMS], f32, isOutput=True)
    outq_ext = nc.declare_dram_parameter("out_q", [1, 1], f32, isOutput=True)

    x_t = x_ext[:].rearrange("(p f) -> p f", p=PARTS)
    outx_t = outx_ext[:].rearrange("(p f) -> p f", p=PARTS)

    with (
        nc.sbuf_tensor([PARTS, FD], f32) as Tt,
        nc.sbuf_tensor([PARTS, FD], f32) as Vt,
        nc.sbuf_tensor([PARTS, FD], f32) as Sf,
        nc.sbuf_tensor([PARTS, FD], f32) as Sg,
        nc.sbuf_tensor([PARTS, 1], f32) as Qp,
        nc.psum_tensor([1, 1], f32) as Pq,
        nc.sbuf_tensor([1, 1], f32) as Qs,
        nc.sbuf_tensor([PARTS, 1], f32) as Dm,
        nc.sbuf_tensor([PARTS, 1], f32) as C3f,
        nc.sbuf_tensor([PARTS, 1], f32) as C3g,
        nc.semaphore("dma_sem") as dma_sem,
        nc.semaphore("act_sem") as act_sem,
        nc.semaphore("vec_sem") as vec_sem,
        nc.semaphore("gp_sem") as gp_sem,
        nc.Block() as block,
    ):
        HP = PARTS // 2
        h0, h1 = slice(0, HP), slice(HP, PARTS)

        @block.sync
        def _(sync):
            sync.dma_start(out=Tt[h0, :], in_=x_t[h0, :]).then_inc(dma_sem, 16)
            sync.wait_ge(vec_sem, 1)
            sync.dma_start(out=outx_t[h0, :], in_=Sf[h0, :]).then_inc(dma_sem, 16)
            sync.wait_ge(act_sem, 2)
            sync.dma_start(out=outq_ext[:], in_=Qs[:]).then_inc(dma_sem, 16)
            sync.wait_ge(dma_sem, 80)

        @block.scalar
        def _(scalar):
            scalar.dma_start(out=Tt[h1, :], in_=x_t[h1, :]).then_inc(dma_sem, 16)
            zero = nc.const_aps.scalar_like(0.0, Dm[:])
            one = nc.const_aps.scalar_like(1.0, Dm[:])
            # chain-start 4th constants (C3 spill operands), written while
            # the input DMA is in flight: out = Copy(c * 1)
            scalar.mul(C3f[:], one, float(cf_t[3]))
            scalar.mul(C3g[:], one, float(cg_t[3]))
            # dummy activation: triggers ACT_TABLE_LOAD for the tanh set
            scalar.activation(
                Dm[:], zero,
                mybir.ActivationFunctionType.Tanh, bias=0.0, scale=1.0,
            )
            scalar.wait_ge(dma_sem, 32)
            scalar.activation(
                Vt[:], Tt[:], mybir.ActivationFunctionType.Tanh,
                bias=0.0, scale=float(ALPHA),
            ).then_inc(act_sem, 1)
            scalar.wait_ge(vec_sem, 1)
            scalar.dma_start(out=outx_t[h1, :], in_=Sf[h1, :]).then_inc(dma_sem, 16)
            # q-sum PSUM -> SBUF; sync's idle HW queue ships it
            scalar.wait_ge(gp_sem, 1)
            scalar.copy(Qs[:], Pq[:]).then_inc(act_sem, 1)

        @block.vector
        def _(vector):
            vector.wait_ge(act_sem, 1)
            # ---- f chain first: its big output DMA posts while g runs ----
            vector._custom_dve(F4, out=Sf[:], in0=Vt[:], in1=C3f[:],
                               s0=float(cf_t[0]), s1=float(cf_t[1]),
                               imm2=float(cf_t[2]))
            for j in range(2):
                vector._custom_dve(
                    S3, out=Sf[:], in0=Sf[:], in1=Vt[:],
                    s0=float(cf_t[4 + 3 * j]), s1=float(cf_t[5 + 3 * j]),
                    imm2=float(cf_t[6 + 3 * j]),
                )
            # final: S = ((S + c2)*v + c1)*v + c0
            vector._custom_dve(
                S2F, out=Sf[:], in0=Sf[:], in1=Vt[:],
                s0=float(cf_t[10]), s1=float(cf_t[11]), imm2=float(cf_t[12]),
            ).then_inc(vec_sem, 1)

            # ---- g chain: degree D_G = 4, one op with sum accumulator ----
            vector._custom_dve(
                F4A, out=Sg[:], accum_out=Qp[:], in0=Vt[:], in1=C3g[:],
                s0=float(cg_t[0]), s1=float(cg_t[1]), imm2=float(cg_t[2]),
            ).then_inc(vec_sem, 1)

        @block.tensor
        def _(tensor):
            # sum the 128 per-partition partials with a ones-matmul:
            # Pq[0,0] = sum_k Qp[k,0] * 1
            tensor.wait_ge(vec_sem, 2)
            tensor.matmul(
                Pq[:], Qp[:], nc.const_aps.tensor(1.0, (PARTS, 1)),
            ).then_inc(gp_sem, 1)

    # raw Bass does not lower wrapper instructions (InstCustomDveAnt) to ISA
    # bytes on its own; Bacc.compile does this, so do it explicitly here.
    mybir.codegen_inst_isa_subclasses(nc)
    return nc


# ---------------------------------------------------------------- kernel
def kernel(**inputs):
    global _LAST_RESULTS
    from concourse.bass_utils import run_bass_kernel_spmd

    weights = {k: np.asarray(v, dtype=np.float32) for k, v in inputs.items()
               if k != "x"}
    x = np.asarray(inputs["x"], dtype=np.float32)
    assert x.shape == (P, B)

    cf64, cg64 = _fit_coeffs(weights)
    # device ordering: highest degree first; g's c_0 stays on the host
    cf_t = tuple(float(np.float32(v)) for v in cf64[::-1])          # c23..c0
    cg_t = tuple(float(np.float32(v)) for v in cg64[:0:-1])         # c12..c1

    nc = _build_graph(cf_t, cg_t)

    in_maps = []
    for i in range(N_CORES):
        shard = np.ascontiguousarray(
            x[i * SHARD_P:(i + 1) * SHARD_P, :]).reshape(-1)
        in_maps.append({"x": shard})

    res = run_bass_kernel_spmd(
        nc, in_maps, core_ids=list(range(N_CORES)),
        trace=bool(os.environ.get("AML_TRACE")),
    )
    _LAST_RESULTS = res

    x_out = np.concatenate(
        [np.asarray(res.results[i]["out_x"], dtype=np.float32)
         for i in range(N_CORES)]
    ).reshape(P * B, 1)

    qsum = np.float64(0.0)
    for i in range(N_CORES):
        qsum += float(np.asarray(res.results[i]["out_q"]).reshape(-1)[0])
    qt = np.float32(qsum / (P * B) + cg64[0])
    return x_out, np.array([qt], dtype=np.float32)
